# revision 45
# baseline (speedup 1.0000x reference)
"""Trainium2 Bass kernel for nn_MemorySystem (cosine-sim attention memory read).

reference:
    x_norm = ||x||_row (B,1); m_norm = ||m||_row (S,1)
    sims = (x @ m^T) / max(x_norm * m_norm^T, 1e-8)
    attn = softmax(8.0 * sims, axis=1)
    out  = attn @ m                       # (B, D)

Sharding: memory_bank rows split across 8 NeuronCores (8192 rows each).
Each core computes, for its shard, the un-normalized softmax numerator
O_c = exp(S_c) @ m_c (B, D) and denominator Z_c = sum_s exp (B,), using the
bounded-score property (|8*cos| <= 8) to skip the max-subtraction entirely.
Per 512-query pass, a ReduceScatter(add) over the [512, 513] bf16 partials
(O|Z) leaves each core with a fully-reduced 64-query slice; it divides O/Z
on-chip and emits those rows. The host reassembles the 8x2 slices.

Changes vs the 355us baseline (to ~290-300us traced / ~27% faster):
  - mm2 (exp @ m) runs in fp8e4 with MatmulPerfMode.DoubleRow: the exp is
    written as fp8 into s-tile PAIR buffers [128, 2, q] and each DoubleRow
    matmul contracts 256 rows at once -> 2x Tensor-engine throughput and
    half the mm2 instruction count. m is kept natural-layout in fp8
    (m_nat); measured end-to-end rel err 8.5e-3 < 2e-2 (emulated 8.8e-3).
  - mm2 pairs are issued two-at-a-time (every 4 s-tiles) to halve the
    ~0.2us bf16<->fp8 PE mode-transition penalty. (Four-at-a-time drains
    the exp pipeline and regresses.)
  - RS partials are bf16 (halves collective bytes; 525KB ReduceScatter).
  - the RS consumption (divide by Z + output DMA) is emitted after ALL
    compute; pass 0's division runs on GpSimd (normalize_recip) because
    the tile scheduler hoists it mid-queue and any RS-dependent wait on
    the in-order DVE queue stalls the whole pipeline (was 33us in the
    f32 baseline, 13us with DVE-side deferral, 0 with GpSimd).
  - x is loaded with two batched 1MB DMAs and kept resident; the DVE
    normalize runs up front for all 8 q-tiles but only pass-0's 4 PE
    transposes gate the loop start — pass-1's are emission-interleaved
    into early pass-0 iterations (the in-order PE queue must never wait
    on the DVE queue mid-loop).
  - fp8 casts of m alternate DVE/ACT to balance engine load during the
    load phase; stage copies alternate ACT/DVE to halve staging latency.

On-chip structure (per core):
  - x is normalized on load (1/||x_q|| folded in) and transposed via TensorE
    to xhatT [d, q] (bf16).
  - m shard is SBUF-resident twice: transposed [d, s] bf16 (lhsT of mm1)
    and natural [s, d] fp8 (rhs of mm2).
  - scores are computed transposed, [s-tile, q], so 8/||m_s|| is a
    per-partition scalar fused into the ACT Exp, and exp^T feeds mm2 as
    the stationary operand directly.
  - row norms: ACT Square (in every ACT table set -> no table thrash with
    Exp) with accum_out and scale=1/8 giving n2/64; rsqrt via DVE
    bit-trick + 2 Newton steps (no ACT Sqrt -> no table swaps).
  - Z is accumulated on DVE (zacc += exp-tile), cross-partition-reduced by
    one ones-matmul per pass; the division happens after the ReduceScatter.
  - chunk 0 of the query loop is emission-interleaved with the m-load loop
    so the in-order PE stream overlaps DMA/cast/norm work with matmuls.
"""

import sys

sys.path.insert(0, "/opt/trn_rl_repo")

import numpy as np
from contextlib import ExitStack

B, S, D = 1024, 65536, 512
NCORES = 8
S_SHARD = S // NCORES  # 8192
P = 128

ST = S_SHARD // P  # 64 s-tiles per core
QT = B // P  # 8 q-tiles
DC = D // P  # 4 d-chunks
NPASS = 2  # q processed in 2 passes of 512
QP = B // NPASS  # 512
QPT = QP // P  # 4 q-tiles per pass
QR = QP // NCORES  # 64 rows of the per-pass ReduceScatter output
NPAIR = ST // 2  # 32 s-tile pairs per pass (DoubleRow mm2)
LAG = 10  # load runs this many s-tiles ahead of chunk-0 compute

MAGIC = 0x5F3759DF

_CACHE = {}


def _build():
    import concourse.bass as bass
    import concourse.tile as tile
    from concourse import bacc, mybir
    from concourse.masks import make_identity

    f32 = mybir.dt.float32
    bf16 = mybir.dt.bfloat16
    fp8 = mybir.dt.float8e4
    u32 = mybir.dt.uint32
    AF = mybir.ActivationFunctionType
    ALU = mybir.AluOpType
    DR = mybir.MatmulPerfMode.DoubleRow

    nc = bacc.Bacc(None, num_devices=NCORES)
    x_ext = nc.declare_dram_parameter("x", [B, D], f32, isOutput=False)
    m_ext = nc.declare_dram_parameter("mem", [S_SHARD, D], f32, isOutput=False)
    out_ext = nc.declare_dram_parameter("out", [B // NCORES, D], f32, isOutput=True)

    with tile.TileContext(nc) as tc, ExitStack() as ctx:
        persist = ctx.enter_context(tc.tile_pool(name="persist", bufs=1))
        loadp = ctx.enter_context(tc.tile_pool(name="load", bufs=11))
        xp = ctx.enter_context(tc.tile_pool(name="xp", bufs=1))
        sqp = ctx.enter_context(tc.tile_pool(name="sqp", bufs=2))
        mbp = ctx.enter_context(tc.tile_pool(name="mbp", bufs=2))
        work = ctx.enter_context(tc.tile_pool(name="work", bufs=4))
        zp = ctx.enter_context(tc.tile_pool(name="zp", bufs=2))
        stp = ctx.enter_context(tc.tile_pool(name="stp", bufs=2))
        finp = ctx.enter_context(tc.tile_pool(name="finp", bufs=2))
        dram = ctx.enter_context(tc.tile_pool(name="dram", bufs=4, space="DRAM"))
        # PSUM: 8 banks total. sc(2) + o2(QPT=4) + tp(2, shared with zt) = 8
        psum_sc = ctx.enter_context(tc.tile_pool(name="psc", bufs=2, space="PSUM"))
        psum_o = ctx.enter_context(tc.tile_pool(name="po", bufs=QPT, space="PSUM"))
        psum_tp = ctx.enter_context(tc.tile_pool(name="ptp", bufs=2, space="PSUM"))
        psum_zt = psum_tp

        # ---- constants ----
        ident_bf = persist.tile([P, P], bf16)
        make_identity(nc, ident_bf[:])
        ones_f32 = persist.tile([P, 1], f32)
        nc.vector.memset(ones_f32[:], 1.0)
        one_f32 = persist.tile([1, 1], f32)
        nc.vector.memset(one_f32[:], 1.0)
        magic_u = persist.tile([P, 1], u32)
        nc.vector.memset(magic_u[:], MAGIC)

        # ---- persistent SBUF tensors ----
        m_nat = persist.tile([P, ST, D], fp8)  # [s%128, s//128, d] fp8 (mm2 rhs)
        mT = persist.tile([P, DC, S_SHARD], bf16)  # [d%128, d//128, s] (mm1 lhsT)
        xhatT = persist.tile([P, DC, B], bf16)  # [d%128, d//128, q]
        n2m = persist.tile([P, ST], f32)  # ||m_s||^2 / 64
        rs_m = persist.tile([P, ST], f32)  # 8 / ||m_s||
        rs_u = persist.tile([P, ST], u32)  # newton scratch (bit-trick y)
        rs_t = persist.tile([P, ST], f32)  # newton scratch t1
        xn2 = persist.tile([P, QT], f32)
        rs_x = persist.tile([P, QT], f32)
        xr_u = persist.tile([P, QT], u32)
        xr_t = persist.tile([P, QT], f32)

        def rsqrt_newton(dst, a, uscr, tscr, n):
            """dst = 1/sqrt(a); all APs [P, n] f32 (uscr u32)."""
            mg = magic_u[:, 0:1]
            if n > 1:
                mg = mg.to_broadcast((P, n))
            nc.vector.tensor_scalar(
                uscr, a.bitcast(u32), 1, None, ALU.logical_shift_right
            )
            nc.vector.tensor_tensor(uscr, mg, uscr, ALU.subtract)
            y = uscr.bitcast(f32)
            for it in range(2):
                out_y = dst if it == 1 else y
                nc.vector.tensor_tensor(tscr, y, y, ALU.mult)
                nc.vector.tensor_tensor(tscr, tscr, a, ALU.mult)
                nc.vector.tensor_scalar(tscr, tscr, -0.5, 1.5, ALU.mult, ALU.add)
                nc.vector.tensor_tensor(out_y, y, tscr, ALU.mult)

        # ---- per-s-tile load step (DMA, norms, casts, transpose) ----
        def load_tile(t):
            mf = loadp.tile([P, D], f32, tag="mf32", name=f"mf_{t}")
            nc.sync.dma_start(out=mf[:], in_=m_ext[t * P : (t + 1) * P, :])
            msq = sqp.tile([P, D], f32, tag="sq", name=f"msq_{t}")
            # scale=1/8: accum collects sum((m/8)^2) = n2/64; rsqrt -> 8/||m||
            nc.scalar.activation(
                out=msq[:], in_=mf[:], func=AF.Square, scale=0.125,
                accum_out=n2m[:, t : t + 1],
            )
            if t % 4 == 3:
                s = slice(t - 3, t + 1)
                rsqrt_newton(rs_m[:, s], n2m[:, s], rs_u[:, s], rs_t[:, s], 4)
            # fp8 natural-layout copy for mm2; alternate engines for balance
            if t % 2 == 0:
                nc.vector.tensor_copy(out=m_nat[:, t, :], in_=mf[:])
            else:
                nc.scalar.activation(
                    out=m_nat[:, t, :], in_=mf[:], func=AF.Copy
                )
            # bf16 scratch for the PE transpose (mm1 needs bf16 precision)
            mb = mbp.tile([P, D], bf16, tag="mb", name=f"mb_{t}")
            nc.vector.tensor_copy(out=mb[:], in_=mf[:])
            mtp = psum_tp.tile([P, DC * P], bf16, tag="tp", name=f"mtp_{t}")
            for c in range(DC):
                nc.tensor.transpose(
                    mtp[:, c * P : (c + 1) * P],
                    mb[:, c * P : (c + 1) * P],
                    ident_bf[:],
                )
            nc.vector.tensor_copy(
                out=mT[:, :, t * P : (t + 1) * P],
                in_=mtp[:].rearrange("p (c q) -> p c q", c=DC),
            )

        loaded = set()

        def load_tile_once(t):
            if t < ST and t not in loaded:
                loaded.add(t)
                load_tile(t)

        # ---- x prep (single DMA pass; xf tiles stay resident) ----
        # x is loaded and squared FIRST: the x chain (dma -> square -> rsqrt
        # -> normalize -> transpose) gates mm1(0), while the m tiles only
        # gate later work. Pass 0 only needs q-tiles 0..3 prepped up front;
        # the rest is emission-interleaved into the early main-loop steps.
        xf_all = xp.tile([P, QT, D], f32, tag="xf", name="xf_all")
        half = QT // 2
        for hx in range(4):
            nc.sync.dma_start(
                out=xf_all[:, hx * 2 : (hx + 1) * 2, :],
                in_=x_ext[hx * 2 * P : (hx + 1) * 2 * P, :].rearrange(
                    "(j p) d -> p j d", p=P
                ),
            )
        xfs = [xf_all[:, j, :] for j in range(QT)]
        for j in range(QT):
            xsq = sqp.tile([P, D], f32, tag="sq", name=f"xsq_{j}")
            nc.scalar.activation(
                out=xsq[:], in_=xfs[j], func=AF.Square,
                accum_out=xn2[:, j : j + 1],
            )
        for u in range(LAG):
            load_tile_once(u)
        # rsqrt + normalize (DVE) for all q-tiles up front; the PE transposes
        # are split: q-tiles 0..3 now (gate mm1(0)), 4..7 interleaved into the
        # main loop once their xhat has long been ready, so the in-order PE
        # queue never waits on the DVE queue mid-loop.
        rsqrt_newton(rs_x[:, 0:half], xn2[:, 0:half], xr_u[:, 0:half],
                     xr_t[:, 0:half], half)
        xhats = {}

        def xmul(j):
            xhat = work.tile([P, D], bf16, tag="xhat", bufs=8, name=f"xhat_{j}")
            nc.vector.tensor_scalar_mul(xhat[:], xfs[j], rs_x[:, j : j + 1])
            xhats[j] = xhat

        def xtrans(j):
            xhat = xhats.pop(j)
            xtp = psum_tp.tile([P, DC * P], bf16, tag="tp", name=f"xtp_{j}")
            for c in range(DC):
                nc.tensor.transpose(
                    xtp[:, c * P : (c + 1) * P],
                    xhat[:, c * P : (c + 1) * P],
                    ident_bf[:],
                )
            nc.vector.tensor_copy(
                out=xhatT[:, :, j * P : (j + 1) * P],
                in_=xtp[:].rearrange("p (c q) -> p c q", c=DC),
            )

        for j in range(QPT):
            xmul(j)
            xtrans(j)
        rsqrt_newton(rs_x[:, half:QT], xn2[:, half:QT], xr_u[:, half:QT],
                     xr_t[:, half:QT], QT - half)
        for j in range(QPT, QT):
            xmul(j)

        # ---- main: scores^T -> exp(fp8, pairs) -> O (DoubleRow) / Z (DVE) ----
        rs_finish = []
        for h in range(NPASS):
            o2 = []
            for j in range(QPT):
                o2.append(psum_o.tile([P, D], f32, tag="o2", name=f"o2_{h}_{j}"))
            zacc = zp.tile([P, QP], f32, tag="zacc", name=f"zacc_{h}")
            nc.gpsimd.memset(zacc[:], 0.0)
            zg = None

            scs = {}
            pts = {}

            def _mm1(t, h=h, scs=scs):
                sc = psum_sc.tile([P, QP], f32, tag="sc", name=f"sc_{h}_{t}")
                for c in range(DC):
                    nc.tensor.matmul(
                        sc[:],
                        mT[:, c, t * P : (t + 1) * P],
                        xhatT[:, c, h * QP : (h + 1) * QP],
                        start=(c == 0),
                        stop=(c == DC - 1),
                    )
                scs[t] = sc

            def _exp(t, h=h, scs=scs, pts=pts, zacc=zacc, zg=zg):
                pr = t // 2
                if t % 2 == 0:
                    pts[pr] = work.tile(
                        [P, 2, QP], fp8, tag="pt", name=f"pt_{h}_{pr}"
                    )
                sc = scs.pop(t)
                slot = pts[pr][:, t % 2, :]
                nc.scalar.activation(
                    out=slot, in_=sc[:], func=AF.Exp, scale=rs_m[:, t : t + 1]
                )
                if zg is not None and t % 2 == 0:
                    nc.gpsimd.tensor_add(zg[:], zg[:], slot)
                else:
                    nc.vector.tensor_add(zacc[:], zacc[:], slot)

            def _mm2(pr, h=h, pts=pts, o2=o2):
                t2 = 2 * pr
                pt = pts.pop(pr)
                for j in range(QPT):
                    nc.tensor.matmul(
                        o2[j][:],
                        pt[:, :, j * P : (j + 1) * P],
                        m_nat[:, t2 : t2 + 2, :],
                        start=(pr == 0),
                        stop=(pr == NPAIR - 1),
                        perf_mode=DR,
                    )

            def _load(u):
                if h == 0:
                    load_tile_once(u)

            _mm1(0)
            for t in range(1, ST):
                _load(t + LAG - 1)
                if h == 0 and QPT <= t < QT:
                    xtrans(t)  # q-tiles 4..7 transposes (xhat long ready)
                _mm1(t)
                _exp(t - 1)
                # two pairs back-to-back every 4 tiles: halves the number of
                # bf16<->fp8 PE mode transitions
                if t >= 7 and t % 4 == 3:
                    _mm2((t - 7) // 2)
                    _mm2((t - 5) // 2)
            _exp(ST - 1)
            _mm2(NPAIR - 2)
            _mm2(NPAIR - 1)

            # cross-partition Z reduce: [1, QP] = ones^T @ zacc (+ zg)
            zsum = psum_zt.tile([1, QP], f32, tag="tp", name=f"zsum_{h}")
            if zg is not None:
                nc.tensor.matmul(zsum[:], ones_f32[:], zacc[:], start=True, stop=False)
                nc.tensor.matmul(zsum[:], ones_f32[:], zg[:], start=False, stop=True)
            else:
                nc.tensor.matmul(zsum[:], ones_f32[:], zacc[:], start=True, stop=True)
            zrow = finp.tile([1, QP], f32, tag="zrow", name=f"zrow_{h}")
            nc.vector.tensor_copy(out=zrow[:], in_=zsum[:])
            ztp = psum_zt.tile([P, QPT], f32, tag="tp", name=f"ztp_{h}")
            for j in range(QPT):
                nc.tensor.transpose(
                    ztp[:, j : j + 1], zrow[0:1, j * P : (j + 1) * P], one_f32[:]
                )

            # stage bf16 [128, QPT, D+1]: cols 0..D-1 = O, col D = Z.
            # Z column first, then per-j O copies (split ACT/DVE) with the
            # partial DMA fired per j as its slice completes — overlaps the
            # DRAM transfer with the remaining staging.
            stage = stp.tile([P, QPT, D + 1], bf16, tag="stage", name=f"stage_{h}")
            nc.vector.tensor_copy(
                out=stage[:, :, D : D + 1],
                in_=ztp[:].rearrange("p (j o) -> p j o", o=1),
            )
            partial = dram.tile(
                [QP, D + 1], bf16, tag="partial", name=f"partial_{h}"
            )
            for j in range(QPT):
                if j % 2 == 0:
                    nc.scalar.activation(
                        out=stage[:, j, 0:D], in_=o2[j][:], func=AF.Copy
                    )
                else:
                    nc.vector.tensor_copy(out=stage[:, j, 0:D], in_=o2[j][:])
                nc.sync.dma_start(
                    out=partial[j * P : (j + 1) * P, :],
                    in_=stage[:, j, :],
                )
            rsout = dram.tile(
                [QR, D + 1], bf16, tag="rsout", name=f"rsout_{h}"
            )
            nc.gpsimd.collective_compute(
                "ReduceScatter",
                mybir.AluOpType.add,
                replica_groups=[list(range(NCORES))],
                ins=[partial[:].opt()],
                outs=[rsout[:].opt()],
            )
            rs_finish.append((h, rsout))

        # RS consumption: divide O/Z and emit output rows. Pass 0's division
        # runs on GpSimd (idle engine) so its RS wait cannot stall the
        # in-order DVE queue mid-kernel (the tile scheduler is free to hoist
        # it); pass 1's runs on DVE at the very end where nothing follows.
        for h, rsout in rs_finish:
            fin = finp.tile([QR, D + 1], bf16, tag="fin", name=f"fin_{h}")
            nc.sync.dma_start(out=fin[:], in_=rsout[:])
            outb = finp.tile([QR, D], f32, tag="outb", name=f"outb_{h}")
            if h == 0:
                finf = finp.tile([QR, D + 1], f32, tag="finf", name=f"finf_{h}")
                nc.gpsimd.tensor_copy(out=finf[:], in_=fin[:])
                nc.gpsimd.normalize_recip(
                    outb[:], finf[:, 0:D], finf[:, D : D + 1]
                )
            else:
                rz = finp.tile([QR, 1], f32, tag="rz", name=f"rz_{h}")
                nc.vector.reciprocal(rz[:], fin[:, D : D + 1])
                nc.vector.tensor_scalar_mul(outb[:], fin[:, 0:D], rz[:])
            nc.sync.dma_start(out=out_ext[h * QR : (h + 1) * QR, :], in_=outb[:])

    nc.compile()
    return nc


def _get_nc():
    if "nc" not in _CACHE:
        _CACHE["nc"] = _build()
    return _CACHE["nc"]


def _run(x, memory_bank, trace=False, **trace_kwargs):
    from concourse.bass_utils import run_bass_kernel_spmd

    nc = _get_nc()
    x = np.ascontiguousarray(np.asarray(x, dtype=np.float32))
    memory_bank = np.ascontiguousarray(np.asarray(memory_bank, dtype=np.float32))
    in_maps = [
        {
            "x": x,
            "mem": np.ascontiguousarray(
                memory_bank[i * S_SHARD : (i + 1) * S_SHARD]
            ),
        }
        for i in range(NCORES)
    ]
    res = run_bass_kernel_spmd(
        nc, in_maps, list(range(NCORES)), trace=trace, **trace_kwargs
    )
    # core i's output rows h*QR..(h+1)*QR hold global q rows h*QP + i*QR + k
    out = np.empty((B, D), dtype=np.float32)
    for i in range(NCORES):
        r = np.asarray(res.results[i]["out"])
        for h in range(NPASS):
            out[h * QP + i * QR : h * QP + (i + 1) * QR] = r[
                h * QR : (h + 1) * QR
            ]
    return out, res


def kernel(x, memory_bank):
    out, _ = _run(x, memory_bank)
    return out


if __name__ == "__main__":
    xs = np.random.randn(B, D).astype(np.float32)
    ms = np.random.randn(S, D).astype(np.float32)
    o = kernel(xs, ms)
    print(o.shape, o.dtype)
